# revision 85
# baseline (speedup 1.0000x reference)
"""Trainium2 Bass kernel for nn_Attention_37495064494240.

Self-contained: takes full (unsharded) numpy inputs, shards batch-wise over
8 NeuronCores, runs a Bass/Tile kernel per core, gathers full outputs.

Per-core dataflow (2 batch elements per core):
  - host: X transposed to XT [H, TW] (feature-major, per-batch token windows
    of stride 640, zero-padded), Wq/bq pre-scaled by 1/sqrt(DH).
  - device: QT/KT = W.T-projections in feature-major layout, V token-major
    (all fp32r matmuls, fp32 PSUM accumulate).
  - per (batch, head-pair): scores in natural [s1, s2] orientation (row-0
    CLS fix via K=1 accumulate-matmuls, max via DVE reduce), exp+row-sums on
    ScalarE (accum_out), normalize -> `weights` output; scores transposed
    [s2, s1] for the ctx matmul (probs^T operand) + sm2; ctx normalized via
    ones-matmul-broadcast reciprocal row-sums; out-projection + bias.
"""

import sys

if "/opt/trn_rl_repo" not in sys.path:
    sys.path.insert(0, "/opt/trn_rl_repo")

import numpy as np

B, S, H = 16, 626, 768
NH, DH = 12, 64
NCORES = 8
BPC = B // NCORES            # batches per core
SW = 640                     # per-batch token stride in XT/QT/KT
TW = BPC * SW                # 1280; batch-1 windows use a 114-wide tail split
WIN = 768                    # matmul rhs window width (626 real + pads)
CH = [(i * 128, min(128, S - i * 128)) for i in range(5)]  # s-chunks

_CACHE = {}

_NPAIRS = NH // 2
_NBATCH = BPC


def _build():
    import concourse.bacc as bacc
    import concourse.mybir as mybir
    import concourse.tile as tile

    f32 = mybir.dt.float32
    f32r = mybir.dt.float32r
    f16 = mybir.dt.float16
    AF = mybir.ActivationFunctionType
    ALU = mybir.AluOpType
    AX = mybir.AxisListType

    nc = bacc.Bacc("TRN2", target_bir_lowering=False, debug=False)

    xt_d = nc.dram_tensor("xt", [H, TW], f32r, kind="ExternalInput").ap()
    wq_d = nc.dram_tensor("wq", [H, H], f32r, kind="ExternalInput").ap()
    wk_d = nc.dram_tensor("wk", [H, H], f32r, kind="ExternalInput").ap()
    wv_d = nc.dram_tensor("wv", [H, H], f32r, kind="ExternalInput").ap()
    wo_d = nc.dram_tensor("wo", [H, H], f16, kind="ExternalInput").ap()
    bqc_d = nc.dram_tensor("bqc", [128, 6], f32, kind="ExternalInput").ap()
    bkc_d = nc.dram_tensor("bkc", [128, 6], f32, kind="ExternalInput").ap()
    bv_d = nc.dram_tensor("bv1", [1, H], f32r, kind="ExternalInput").ap()
    bo_d = nc.dram_tensor("bo1", [1, H], f32r, kind="ExternalInput").ap()
    m25_d = nc.dram_tensor("m25", [1, TW], f32r, kind="ExternalInput").ap()
    ones_d = nc.dram_tensor("ones", [1, 128], f32r, kind="ExternalInput").ap()
    z48_d = nc.dram_tensor("z48", [1, 4 * NH], f32r, kind="ExternalInput").ap()
    id_d = nc.dram_tensor("ident", [128, 128], f32, kind="ExternalInput").ap()

    out_d = nc.dram_tensor("out", [BPC * S, H], f32, kind="ExternalOutput").ap()
    wts_d = nc.dram_tensor("wts", [BPC, NH, S, S], f32, kind="ExternalOutput").ap()
    sm2_d = nc.dram_tensor("sm2", [BPC * NH, S], f32, kind="ExternalOutput").ap()

    with tile.TileContext(nc) as tc:
        with tc.tile_pool(name="persist", bufs=1) as pers:
            ident = pers.tile([128, 128], f32, tag="ident")
            nc.gpsimd.dma_start(ident[:], id_d)
            ones = pers.tile([1, 128], f32r, tag="ones")
            nc.gpsimd.dma_start(ones[:], ones_d)
            m25 = pers.tile([1, TW], f32r, tag="m25")
            nc.gpsimd.dma_start(m25[:], m25_d)
            bqc = pers.tile([128, 6], f32, tag="bqc")
            nc.gpsimd.dma_start(bqc[:], bqc_d)
            bkc = pers.tile([128, 6], f32, tag="bkc")
            nc.gpsimd.dma_start(bkc[:], bkc_d)
            bv1 = pers.tile([1, H], f32r, tag="bv1")
            nc.gpsimd.dma_start(bv1[:], bv_d)
            bo1 = pers.tile([1, H], f32r, tag="bo1")
            nc.gpsimd.dma_start(bo1[:], bo_d)
            mxall = pers.tile([1, 4 * NH], f32r, tag="mxall")
            nc.gpsimd.dma_start(mxall[0:1, :], z48_d)

            qt = [pers.tile([128, TW], f32r, tag=f"qt{i}", name=f"qt{i}") for i in range(6)]
            kt = [pers.tile([128, TW], f32r, tag=f"kt{i}", name=f"kt{i}") for i in range(6)]
            vt = [pers.tile([128, H], f16, tag=f"vt{i}", name=f"vt{i}") for i in range(10)]
            wo = [pers.tile([128, H], f16, tag=f"wo{i}", name=f"wo{i}") for i in range(6)]


            # ---------------- attention pools (allocated first so their
            # SBUF/PSUM space is disjoint from the projection staging pool,
            # letting the scheduler overlap projections with attention) ----
            with tc.tile_pool(name="work", bufs=1) as wk_p, \
                 tc.tile_pool(name="sml", bufs=2) as sml, \
                 tc.tile_pool(name="psS", bufs=2, space="PSUM") as psS, \
                 tc.tile_pool(name="psC", bufs=1, space="PSUM") as psC, \
                 tc.tile_pool(name="wx", bufs=1) as wx:
                xts = [wx.tile([128, TW], f32r, tag=f"xt{k}", name=f"xtt{k}") for k in range(6)]
                for k in range(6):
                    eng = (nc.scalar, nc.gpsimd)[k % 2]
                    eng.dma_start(xts[k][:], xt_d[128 * k:128 * k + 128, :])
                nc.scalar.dma_start(ident[:], id_d)
                nc.gpsimd.dma_start(ones[:], ones_d)
                nc.scalar.dma_start(m25[:], m25_d)
                nc.gpsimd.dma_start(bqc[:], bqc_d)
                nc.scalar.dma_start(bkc[:], bkc_d)
                nc.gpsimd.dma_start(bv1[:], bv_d)
                nc.scalar.dma_start(bo1[:], bo_d)
                nc.gpsimd.dma_start(mxall[0:1, :], z48_d)

                h_splits = [(0, 512), (512, H)]
                a_splits = [(0, 512), (512, WIN)]
                b_splits = [(0, 512)]
                w_splits = [(0, 512), (512, WIN)]
                w_splits1 = [(0, 512), (512, S)]

                # fine-grained weight tiles: 18 tags of [128, 256] (2 f-chunks
                # per group), reused Wv -> Wq -> Wk so each reload only waits
                # on the readers of its own 2 f-chunks
                def load_wset(w_d, nm, groups=range(3), engs=None):
                    tiles = {}
                    for g in groups:
                        eng = nc.sync if engs is None else engs[g]
                        for k in range(6):
                            t = wx.tile([128, 256], f32r, tag=f"wg{k}_{g}",
                                        name=f"{nm}{k}_{g}")
                            eng.dma_start(
                                t[:], w_d[128 * k:128 * k + 128,
                                          256 * g:256 * g + 256])
                            tiles[(k, g)] = t
                    return tiles

                def wslice(tiles, k, f):
                    g, fh = divmod(f, 2)
                    return tiles[(k, g)][:, 128 * fh:128 * fh + 128]

                # ---- V first (token-major, fp16 out) ----
                wvt = load_wset(wv_d, "wv",
                                engs=[nc.sync, nc.scalar, nc.gpsimd])

                # V rhs needs Wv rows [k-chunk] x cols [c0:c1] -- the fine
                # tiles split the f dim, so V matmuls iterate (k, g) pairs
                def emit_v(t):
                    vb, vi = divmod(t, 5)
                    off = SW * vb + CH[vi][0]
                    ni = CH[vi][1]
                    pv = psS.tile([128, WIN], f32, tag="ps", name="pv")
                    for g in range(3):
                        c0 = 256 * g
                        for k in range(6):
                            nc.tensor.matmul(
                                pv[:ni, c0:c0 + 256],
                                xts[k][:, off:off + ni],
                                wvt[(k, g)][:, 0:256],
                                start=(k == 0), stop=False)
                        nc.tensor.matmul(
                            pv[:ni, c0:c0 + 256], ones[0:1, 0:ni],
                            bv1[0:1, c0:c0 + 256],
                            start=False, stop=True, skip_group_check=True)
                    nc.vector.tensor_copy(vt[t][:ni, :], pv[:ni, 0:H])

                for t in range(10):
                    emit_v(t)

                for k in range(6):
                    nc.gpsimd.dma_start(wo[k][:], wo_d[128 * k:128 * k + 128, :])

                # ---- QT (all f-chunks), feature-major ----
                wqt = load_wset(wq_d, "wq")

                def emit_projT(wt, bias_col, qt_out, f):
                    pa = psS.tile([128, WIN], f32, tag="ps", name="pa")
                    for (c0, c1) in a_splits:
                        for k in range(6):
                            nc.tensor.matmul(
                                pa[:, c0:c1], wslice(wt, k, f),
                                xts[k][:, c0:c1],
                                start=(k == 0), stop=(k == 5))
                    nc.vector.tensor_scalar(
                        qt_out[f][:, 0:WIN], pa[:], bias_col[:, f:f + 1],
                        None, op0=ALU.add)
                    pb = psC.tile([128, TW - WIN], f32, tag=f"cx{f % 2}",
                                  name="pb")
                    for (c0, c1) in b_splits:
                        for k in range(6):
                            nc.tensor.matmul(
                                pb[:, c0:c1], wslice(wt, k, f),
                                xts[k][:, WIN + c0:WIN + c1],
                                start=(k == 0), stop=(k == 5))
                    nc.vector.tensor_scalar(
                        qt_out[f][:, WIN:TW], pb[:], bias_col[:, f:f + 1],
                        None, op0=ALU.add)

                for f in range(6):
                    emit_projT(wqt, bqc, qt, f)

                # ---- KT group-0 weights; remaining groups + f-chunks are
                # emitted inside the pair loop so attention starts early ----
                wkt = load_wset(wk_d, "wk", groups=[0])

                def emit_row0(rb, j0):
                    rwin = SW * rb
                    r0s = {}
                    for hh0 in (0, 1):
                        po0 = 64 * hh0
                        r0 = psS.tile([1, WIN], f32, tag="ps", name="r0")
                        for (c0, c1) in (w_splits if rb == 0 else w_splits1):
                            nc.tensor.matmul(
                                r0[0:1, c0:c1],
                                qt[j0][po0:po0 + 64, rwin:rwin + 1],
                                kt[j0][po0:po0 + 64, rwin + c0:rwin + c1],
                                start=True, stop=True)
                        r0s[hh0] = r0
                    for hh0 in (0, 1):
                        h0 = 2 * j0 + hh0
                        nc.vector.tensor_reduce(
                            mxall[0:1, 2 * h0 + 2 * NH * rb:
                                  2 * h0 + 2 * NH * rb + 1],
                            r0s[hh0][0:1, 0:S], AX.X, ALU.max)

                emit_projT(wkt, bkc, kt, 0)
                emit_row0(0, 0)
                deferred = [(0, j0) for j0 in range(1, _NPAIRS)]
                deferred1 = [(rb, j0) for rb in range(1, _NBATCH)
                             for j0 in range(_NPAIRS)]

                # ---------------- attention ----------------
                s_splits = [(0, 512), (512, S)]

                def emit_outproj_chunk(b, ctxbt, i):
                    o, nI = CH[i]
                    pot = psS.tile([128, WIN], f32, tag="ps", name="pot")
                    for (c0, c1) in h_splits:
                        for jf in range(_NPAIRS):
                            nc.tensor.matmul(
                                pot[:nI, c0:c1], ctxbt[jf][:, o:o + nI],
                                wo[jf][:, c0:c1],
                                start=(jf == 0), stop=False)
                        nc.tensor.matmul(
                            pot[:nI, c0:c1], ones[0:1, 0:nI],
                            bo1[0:1, c0:c1],
                            start=(_NPAIRS == 0), stop=True,
                            skip_group_check=True)
                    osb = wk_p.tile([128, H], f32, tag="osb", bufs=3)
                    nc.vector.tensor_copy(osb[:nI, :], pot[:nI, 0:H])
                    nc.gpsimd.dma_start(
                        out_d[S * b + o:S * b + o + nI, :], osb[:nI, :])

                outproj_q = []

                def emit_outproj(b, ctxbt):
                    for i in range(5):
                        emit_outproj_chunk(b, ctxbt, i)


                def emit_tail(tb, tj, eTs, rf, acc, rcs, ctxbt):
                    rbc, pcs = {}, {}
                    # sm2 rows: stage per head, store via SWDGE
                    for hh in (0, 1):
                        s2r = wk_p.tile([1, S], f32, tag="s2r",
                                        name="s2r", bufs=1)
                        nc.vector.reciprocal(
                            rcs[hh][0:1, 0:1], acc[hh][0:1, 0:1])
                        nc.vector.tensor_scalar(
                            s2r[0:1, :], eTs[hh][0][0:1, :],
                            rcs[hh][0:1, 0:1], None, op0=ALU.mult)
                        r = tb * NH + 2 * tj + hh
                        nc.gpsimd.dma_start(sm2_d[r:r + 1, :], s2r[0:1, :])

                    # broadcast recips across partitions via ones-matmul
                    for hh in (0, 1):
                        pbc = psC.tile([64, WIN], f32, tag=f"cx{hh}",
                                       name="pbc")
                        for (c0, c1) in s_splits:
                            nc.tensor.matmul(
                                pbc[0:64, c0:c1], ones[0:1, 0:64],
                                rf[hh][0:1, c0:c1], start=True, stop=True)
                        rbc[hh] = wk_p.tile([64, S], f32, tag=f"rbc{hh}",
                                            name=f"rbc{hh}", bufs=1)
                        nc.vector.tensor_copy(rbc[hh][:], pbc[0:64, 0:S])

                    # ctx matmuls (per-head psum, dst partition 0)
                    for hh in (0, 1):
                        pcs[hh] = psC.tile([64, S], f32, tag=f"cx{hh}",
                                           name=f"pc{hh}")
                    for k, (ko, kI) in enumerate(CH):
                        v_ = vt[5 * tb + k]
                        for hh in (0, 1):
                            h = 2 * tj + hh
                            for (c0, c1) in s_splits:
                                nc.tensor.matmul(
                                    pcs[hh][0:64, c0:c1],
                                    v_[:kI, 64 * h:64 * h + 64],
                                    eTs[hh][k][:kI, c0:c1],
                                    start=(k == 0), stop=(k == 4))
                    cb = pers.tile([128, S], f16, tag=f"cb{tj}",
                                   name=f"cb{tj}")
                    for hh in (0, 1):
                        po = 64 * hh
                        nc.vector.tensor_tensor(
                            cb[po:po + 64, :], pcs[hh][0:64, :],
                            rbc[hh][:], op=ALU.mult)
                    ctxbt.append(cb)

                prev_ctx = None
                pending = None
                for b in range(_NBATCH):
                    win = SW * b
                    wsp = w_splits if b == 0 else w_splits1
                    ctxbt = []
                    for it_ in deferred + deferred1:
                        emit_row0(*it_)
                    deferred, deferred1 = [], []
                    # previous batch's out-projection overlaps this batch
                    if prev_ctx is not None:
                        emit_outproj(b - 1, prev_ctx)
                    for j in range(_NPAIRS):
                        mx, rs, rec, acc, rcs, rf = {}, {}, {}, {}, {}, {}
                        eTs = {0: [], 1: []}
                        eS, rbc, pcs = {}, {}, {}
                        for hh in (0, 1):
                            mc = 2 * (2 * j + hh) + 2 * NH * b
                            mx[hh] = mxall[0:1, mc:mc + 2]
                            rs[hh] = sml.tile([128, 8], f32, tag=f"rs{hh}", name=f"rs{hh}")
                            rec[hh] = sml.tile([128, 8], f32, tag=f"rec{hh}", name=f"rec{hh}")
                            acc[hh] = sml.tile([128, 1], f32, tag=f"acc{hh}", name=f"acc{hh}")
                            rcs[hh] = sml.tile([1, 1], f32, tag=f"rcs{hh}", name=f"rcs{hh}")
                            rf[hh] = sml.tile([1, S], f32r, tag=f"rf{hh}", name=f"rf{hh}", bufs=1)

                        # ---- natural orientation (staggered MM/exp) ----
                        def snat_mm(i, hh):
                            o, nI = CH[i]
                            po = 64 * hh
                            pss = psS.tile([128, WIN], f32, tag="ps",
                                           name="pss")
                            for (c0, c1) in wsp:
                                nc.tensor.matmul(
                                    pss[:nI, c0:c1],
                                    qt[j][po:po + 64, win + o:win + o + nI],
                                    kt[j][po:po + 64, win + c0:win + c1],
                                    start=True, stop=True)
                            return pss

                        def snat_exp(i, hh, pss):
                            nI = CH[i][1]
                            t = wk_p.tile([128, S], f32, tag="expS",
                                          name="expS", bufs=8)
                            eS[(i, hh)] = t
                            nc.scalar.activation(
                                t[:nI, :], pss[:nI, 0:S], AF.Exp,
                                accum_out=rs[hh][:nI, i:i + 1])
                            nc.vector.reciprocal(rec[hh][:nI, i:i + 1],
                                                 rs[hh][:nI, i:i + 1])

                        p0 = {hh: snat_mm(0, hh) for hh in (0, 1)}
                        for hh in (0, 1):
                            for (c0, c1) in wsp:
                                nc.tensor.matmul(
                                    p0[hh][0:1, c0:c1], mx[hh][0:1, 0:1],
                                    m25[0:1, win + c0:win + c1],
                                    start=False, stop=True,
                                    skip_group_check=True)
                        for hh in (0, 1):
                            snat_exp(0, hh, p0[hh])
                        prev = {}
                        for i in range(1, 5):
                            for hh in (0, 1):
                                prev[hh] = snat_mm(i, hh)
                            for hh in (0, 1):
                                snat_exp(i, hh, prev[hh])

                        # normalize + per-chunk probs stores (DVE + SWDGE)
                        for hh in (0, 1):
                            h = 2 * j + hh
                            for i, (o, nI) in enumerate(CH):
                                t = eS[(i, hh)]
                                nc.vector.tensor_scalar(
                                    t[:nI, :], t[:nI, :],
                                    rec[hh][:nI, i:i + 1], None, op0=ALU.mult)
                                nc.gpsimd.dma_start(
                                    wts_d[b, h, o:o + nI, :], t[:nI, :])

                        # next pair's KT chunk (after this pair's natural
                        # matmuls so ACT is already fed)
                        if b == 0 and j + 1 < _NPAIRS:
                            g_ = (j + 1) // 2
                            if (j + 1) % 2 == 0:
                                for k_ in range(6):
                                    t_ = wx.tile([128, 256], f32r,
                                                 tag=f"wg{k_}_{g_}",
                                                 name=f"wk{k_}_{g_}")
                                    nc.sync.dma_start(
                                        t_[:],
                                        wk_d[128 * k_:128 * k_ + 128,
                                             256 * g_:256 * g_ + 256])
                                    wkt[(k_, g_)] = t_
                            emit_projT(wkt, bkc, kt, j + 1)
                        # drain deferred row0 precomputes (r0(0, j+1) needs
                        # the KT chunk just emitted; r0(1, j) needs KT[j])
                        if b == 0:
                            if deferred:
                                emit_row0(*deferred.pop(0))
                            if deferred1:
                                emit_row0(*deferred1.pop(0))
                        # previous pair's tail (ctx before this pair's sT)
                        if pending is not None:
                            emit_tail(**pending)
                            pending = None

                        # ---- transposed orientation (staggered) ----
                        def st_mm(i2, hh):
                            o2, mI = CH[i2]
                            po = 64 * hh
                            pst = psS.tile([128, WIN], f32, tag="ps",
                                           name="pst")
                            for (c0, c1) in wsp:
                                nc.tensor.matmul(
                                    pst[:mI, c0:c1],
                                    kt[j][po:po + 64, win + o2:win + o2 + mI],
                                    qt[j][po:po + 64, win + c0:win + c1],
                                    start=True, stop=True)
                            nc.tensor.matmul(
                                pst[:mI, 0:2],
                                m25[0:1, win + o2:win + o2 + mI],
                                mx[hh][0:1, 0:2],
                                start=False, stop=True, skip_group_check=True)
                            return pst

                        def st_exp(i2, hh, pst):
                            mI = CH[i2][1]
                            eT = wk_p.tile([128, S], f16,
                                           tag=f"eT{i2}_{hh}",
                                           name=f"eT{i2}_{hh}")
                            if i2 == 0:
                                nc.scalar.activation(
                                    eT[:mI, :], pst[:mI, 0:S], AF.Exp,
                                    accum_out=acc[hh][:mI, 0:1])
                            else:
                                nc.scalar.activation(
                                    eT[:mI, :], pst[:mI, 0:S], AF.Exp)
                            eTs[hh].append(eT)

                        t0 = {hh: st_mm(0, hh) for hh in (0, 1)}

                        # row-sum transposes to free layout + reciprocal
                        # (row sums complete; runs while sT matmuls continue)
                        for hh in (0, 1):
                            rsF = psC.tile([1, WIN], f32, tag=f"cx{hh}",
                                           name="rsF")
                            for i, (o, nI) in enumerate(CH):
                                nc.tensor.transpose(
                                    rsF[0:1, o:o + nI], rs[hh][:nI, i:i + 1],
                                    ident[:nI, :nI])
                            with nc.allow_low_precision(reason="f32r recip"):
                                nc.vector.reciprocal(
                                    rf[hh][0:1, 0:S], rsF[0:1, 0:S])

                        for hh in (0, 1):
                            st_exp(0, hh, t0[hh])
                        tprev = {}
                        for i2 in range(1, 5):
                            for hh in (0, 1):
                                tprev[hh] = st_mm(i2, hh)
                            for hh in (0, 1):
                                st_exp(i2, hh, tprev[hh])

                        # defer this pair's tail to overlap the next pair
                        pending = dict(tb=b, tj=j, eTs=eTs, rf=rf, acc=acc,
                                       rcs=rcs, ctxbt=ctxbt)

                    if pending is not None:
                        emit_tail(**pending)
                        pending = None
                    prev_ctx = ctxbt
                emit_outproj(_NBATCH - 1, prev_ctx)


    nc.compile()
    return nc


def _get_module():
    if "nc" not in _CACHE:
        _CACHE["nc"] = _build()
    return _CACHE["nc"]


def _prep_in_maps(hidden_states, mask, Wq, bq, Wk, bk, Wv, bv, Wo, bo):
    hs = np.asarray(hidden_states, dtype=np.float32)
    mask = np.asarray(mask, dtype=np.float32)
    scale = 1.0 / np.sqrt(np.float32(DH))
    wq_s = np.ascontiguousarray(np.asarray(Wq, np.float32) * scale)
    bq_s = np.asarray(bq, np.float32) * scale
    wk_n = np.ascontiguousarray(np.asarray(Wk, np.float32))
    wv_n = np.ascontiguousarray(np.asarray(Wv, np.float32))
    wo_n = np.ascontiguousarray(np.asarray(Wo, np.float16))
    bqc = np.ascontiguousarray(bq_s.reshape(6, 128).T)
    bkc = np.ascontiguousarray(np.asarray(bk, np.float32).reshape(6, 128).T)
    bv1 = np.asarray(bv, np.float32).reshape(1, H)
    bo1 = np.asarray(bo, np.float32).reshape(1, H)
    ones = np.ones((1, 128), np.float32)
    ident = np.eye(128, dtype=np.float32)

    # mask bias indicator: 0.25 where mask_full < 0.5 (slot 0 forced 0 -> 0.25)
    mask_full = np.concatenate(
        [np.zeros((B, 1), np.float32), mask], axis=1)  # [B, S]
    m25_full = np.where(mask_full < 0.5, np.float32(0.25), np.float32(0.0))

    in_maps = []
    for c in range(NCORES):
        xt = np.zeros((H, TW), np.float32)
        m25 = np.zeros((1, TW), np.float32)
        for b in range(BPC):
            g = BPC * c + b
            xt[:, SW * b:SW * b + S] = hs[g].T
            m25[0, SW * b:SW * b + S] = m25_full[g]
        in_maps.append({
            "xt": xt, "wq": wq_s, "wk": wk_n, "wv": wv_n, "wo": wo_n,
            "bqc": bqc, "bkc": bkc, "bv1": bv1, "bo1": bo1,
            "m25": m25, "ones": ones, "ident": ident,
            "z48": np.zeros((1, 4 * NH), np.float32),
        })
    return in_maps


def kernel(hidden_states, mask, Wq, bq, Wk, bk, Wv, bv, Wo, bo):
    from concourse import bass_utils

    in_maps = _prep_in_maps(hidden_states, mask, Wq, bq, Wk, bk, Wv, bv,
                            Wo, bo)
    nc = _get_module()
    res = bass_utils.run_bass_kernel_spmd(
        nc, in_maps, core_ids=list(range(NCORES)))

    out = np.empty((B, S, H), np.float32)
    weights = np.empty((B, NH, S, S), np.float32)
    sm2 = np.empty((B, NH, S), np.float32)
    for c in range(NCORES):
        r = res.results[c]
        out[BPC * c:BPC * c + BPC] = r["out"].reshape(BPC, S, H)
        weights[BPC * c:BPC * c + BPC] = r["wts"]
        sm2[BPC * c:BPC * c + BPC] = r["sm2"].reshape(BPC, NH, S)
    return out, weights, sm2
